# revision 1
# baseline (speedup 1.0000x reference)
"""NVFP4 fake-quant SwiGLU MLP on 8 Trainium2 NeuronCores.

Sharding: data-parallel over tokens for the matmuls (each core computes 1024
of the 8192 tokens end-to-end); weight *quantization* is sharded Megatron-style
(each core fake-quants 1/8 of each weight) and the quantized bf16 weights are
AllGathered. No other collective is needed: the final output is token-sharded
and concatenated on the host.

Math: fake-quant values q*sc8 are exactly representable in bf16 (q: 2 sig
bits, sc8: e4m3fn 4 sig bits), so all three matmuls run at bf16 PE peak and
the global scales 1/(gs_a*gs_w) are applied to the f32 outputs. e2m1 and
e4m3fn round-to-nearest are computed with custom DVE ops (Veltkamp splitting
for the normal ranges + magic-constant fixed-point rounds for the subnormal
ranges).
"""
import numpy as np

import concourse.bass as bass
import concourse.mybir as mybir
import concourse.tile as tile
from concourse import bacc
from concourse.bass_utils import run_bass_kernel_spmd
from concourse.dve_spec import (
    Spec, Src0, Src1, C0, C1, C2, C3, One, Zero, lower, maxx, minn, select, sq,
    _has_src1, _spill_c3_to_src1,
)
import concourse.dve_ops as dve_ops_mod
from concourse.dve_ops import DveOp, OPS
from concourse.dve_uop import DveOpSpec

F32 = mybir.dt.float32
BF16 = mybir.dt.bfloat16
ALU = mybir.AluOpType
AX = mybir.AxisListType
AF = mybir.ActivationFunctionType

B, S, H, I = 4, 2048, 1024, 4096
NCORES = 8
T = B * S
T_LOC = T // NCORES      # 1024 tokens per core
I_SH = I // NCORES       # 512  gate/up rows per core (quant shard)
HO_SH = H // NCORES      # 128  down rows per core (quant shard)

VELT_E2M1 = float(2**22 + 1)
MAGIC_E2M1 = float(3 * 2**21)
VELT_E4M3 = float(2**20 + 1)
MAGIC_E4M3 = float(2**14)
TH_E4M3 = float(2**-6)

# ---------------------------------------------------------------- custom ops


def _register(name, spec, subdim=False):
    for op in OPS:
        if op.name == name:
            return op
    idx = len(OPS)
    opcode = dve_ops_mod._CUSTOM_DVE_ROW_BASE + idx
    assert opcode < 0x20, "custom DVE row overflow"
    shas = {}
    for ver in ("v3", "v4"):
        shas[ver] = DveOpSpec(
            name=name, opcode=opcode, uops=lower(spec, ver=ver),
            rd1_en=_has_src1(spec),
        ).sha(ver)
    op = DveOp(name, spec, subdim=subdim, uops_sha=shas)
    OPS.append(op)
    dve_ops_mod._SUB_OPCODE_FOR_NAME[name] = opcode
    dve_ops_mod.CUSTOM_DVE_SPECS[name] = spec
    return op


def _ref_scale_clip(in0, in1, s0, s1, imm2):
    m = (in0.astype(np.float32) * in1.astype(np.float32)).astype(np.float32)
    return np.minimum(np.maximum(m, np.float32(-s0)), np.float32(s0))


def _ref_subnorm_sel(in0, in1, s0, s1, imm2):
    t = in0.astype(np.float32)
    u = (t + np.float32(s0)).astype(np.float32)
    v = (u - np.float32(s0)).astype(np.float32)
    return np.where((t * t).astype(np.float32) < 1.0, v, t).astype(np.float32)


def _ref_velt_scale(in0, in1, s0, s1, imm2):
    t = in0.astype(np.float32)
    gam = (t * np.float32(s0)).astype(np.float32)
    delta = (t - gam).astype(np.float32)
    hi = (gam + delta).astype(np.float32)
    return (hi * in1.astype(np.float32)).astype(np.float32)


def _ref_e4m3(in0, in1, s0, s1, imm2):
    cap = in1.reshape(in1.shape[0], 1).astype(np.float32)
    t = np.minimum(in0.astype(np.float32), cap)
    gam = (t * np.float32(s0)).astype(np.float32)
    delta = (t - gam).astype(np.float32)
    hi = (gam + delta).astype(np.float32)
    u = (t + np.float32(s1)).astype(np.float32)
    v = (u - np.float32(s1)).astype(np.float32)
    return np.where(t < np.float32(imm2), v, hi).astype(np.float32)


_m = Src0 * Src1
OP_SCALE_CLIP = _register(
    "NVFP4_SCALE_CLIP_ANT",
    Spec(body=minn(maxx(_m, Zero - C0), C0), reference=_ref_scale_clip),
)
_u = Src0 + C0
_v = _u - C0
OP_E2M1_SUBNORM = _register(
    "NVFP4_E2M1_SUBNORM_ANT",
    Spec(body=select(sq(Src0) < One, _v, Src0), reference=_ref_subnorm_sel),
)
_gam = Src0 * C0
_hi = _gam + (Src0 - _gam)
OP_VELT_SCALE = _register(
    "NVFP4_VELT_SCALE_ANT",
    Spec(body=_hi * Src1, reference=_ref_velt_scale),
)
_t = minn(Src0, C3)
_gam4 = _t * C0
_hi4 = _gam4 + (_t - _gam4)
_v4 = (_t + C1) - C1
OP_E4M3 = _register(
    "NVFP4_E4M3_ANT",
    Spec(body=_spill_c3_to_src1(select(_t < C2, _v4, _hi4)), reference=_ref_e4m3),
)


def quantize_tile(nc, work, src_f32, out_bf16, n, gs, c448_col):
    """src_f32 [128, n] (true values, 16-blocks on free dim) -> out_bf16 = q*sc8."""
    nblk = n // 16
    gs = float(np.float32(gs))
    src3 = src_f32.rearrange("p (b s) -> p b s", s=16)
    amax = work.tile([128, nblk], F32, tag="q_amax")
    nc.vector.tensor_reduce(
        out=amax[:], in_=src3, axis=AX.X, op=ALU.max, apply_absolute_value=True
    )
    t1 = work.tile([128, nblk], F32, tag="q_t1")
    nc.vector.tensor_scalar(
        out=t1[:], in0=amax[:], scalar1=float(np.float32(1.0 / 6.0)), scalar2=gs,
        op0=ALU.mult, op1=ALU.mult,
    )
    sc8 = work.tile([128, nblk], F32, tag="q_sc8")
    nc.vector._custom_dve(
        OP_E4M3, out=sc8[:], in0=t1[:], in1=c448_col,
        s0=VELT_E4M3, s1=MAGIC_E4M3, imm2=TH_E4M3,
    )
    r = work.tile([128, nblk], F32, tag="q_r")
    nc.vector.reciprocal(r[:], sc8[:])
    r2 = work.tile([128, nblk], F32, tag="q_r2")
    nc.vector.tensor_scalar(
        out=r2[:], in0=r[:], scalar1=gs, scalar2=1e38,
        op0=ALU.mult, op1=ALU.min,
    )
    cc = work.tile([128, n], F32, tag="q_cc")
    cc3 = cc[:].rearrange("p (b s) -> p b s", s=16)
    r2b = r2[:].unsqueeze(-1).broadcast_to([128, nblk, 16])
    nc.vector._custom_dve(OP_SCALE_CLIP, out=cc3, in0=src3, in1=r2b, s0=6.0)
    pp = work.tile([128, n], F32, tag="q_pp")
    nc.vector._custom_dve(OP_E2M1_SUBNORM, out=pp[:], in0=cc[:], s0=MAGIC_E2M1)
    sc8b = sc8[:].unsqueeze(-1).broadcast_to([128, nblk, 16])
    out3 = out_bf16.rearrange("p (b s) -> p b s", s=16)
    pp3 = pp[:].rearrange("p (b s) -> p b s", s=16)
    nc.vector._custom_dve(OP_VELT_SCALE, out=out3, in0=pp3, in1=sc8b, s0=VELT_E2M1)


# ---------------------------------------------------------------- program


def build_program(gs_x, gs_gw, gs_uw, gs_dw, gs_h):
    gs_x, gs_gw, gs_uw, gs_dw, gs_h = (
        np.float32(gs_x), np.float32(gs_gw), np.float32(gs_uw),
        np.float32(gs_dw), np.float32(gs_h),
    )
    s_gate = float(np.float32(1.0) / np.float32(gs_x * gs_gw))
    s_up = float(np.float32(1.0) / np.float32(gs_x * gs_uw))
    s_down = float(np.float32(1.0) / np.float32(gs_h * gs_dw))

    nc = bacc.Bacc("TRN2", num_devices=NCORES, debug=False)
    x_in = nc.dram_tensor("x_slice", [T_LOC, H], F32, kind="ExternalInput")
    gw_in = nc.dram_tensor("gw_slice", [I_SH, H], F32, kind="ExternalInput")
    uw_in = nc.dram_tensor("uw_slice", [I_SH, H], F32, kind="ExternalInput")
    dw_in = nc.dram_tensor("dw_slice", [HO_SH, I], F32, kind="ExternalInput")
    out_d = nc.dram_tensor("out_slice", [T_LOC, H], F32, kind="ExternalOutput")

    RG = [list(range(NCORES))]

    with tile.TileContext(nc) as tc:
        with (
            tc.tile_pool(name="dram", bufs=1, space="DRAM") as dpool,
            tc.tile_pool(name="const", bufs=1) as cpool,
            tc.tile_pool(name="xt", bufs=1) as xtpool,
        ):
            gwq_loc = dpool.tile([I_SH, H], BF16)
            uwq_loc = dpool.tile([I_SH, H], BF16)
            dwq_loc = dpool.tile([HO_SH, I], BF16)
            gwq_g = dpool.tile([I, H], BF16, addr_space="Shared")
            uwq_g = dpool.tile([I, H], BF16, addr_space="Shared")
            dwq_g = dpool.tile([H, I], BF16, addr_space="Shared")
            xq_d = dpool.tile([T_LOC, H], BF16)
            hq_d = dpool.tile([T_LOC, I], BF16)

            c448 = cpool.tile([128, 1], F32)
            nc.vector.memset(c448[:], 448.0)

            # xqsT[h-tile][128h, tok]  (resident through phase C)
            xqsT = xtpool.tile([128, H // 128, T_LOC], BF16)

            # ---- Phase A: quantize own weight shards, allgather bf16
            with (
                tc.tile_pool(name="wraw", bufs=2) as wraw,
                tc.tile_pool(name="wq", bufs=2) as wqp,
                tc.tile_pool(name="workA", bufs=2) as workA,
            ):
                for src, dst, rows, cols, gsw in (
                    (gw_in, gwq_loc, I_SH, H, gs_gw),
                    (uw_in, uwq_loc, I_SH, H, gs_uw),
                    (dw_in, dwq_loc, HO_SH, I, gs_dw),
                ):
                    for r0 in range(0, rows, 128):
                        wt = wraw.tile([128, cols], F32, tag="wraw")
                        nc.sync.dma_start(wt[:], src[r0:r0 + 128, :])
                        wq = wqp.tile([128, cols], BF16, tag="wq")
                        quantize_tile(nc, workA, wt[:], wq[:], cols, gsw, c448[:])
                        nc.sync.dma_start(dst[r0:r0 + 128, :], wq[:])

            for loc, gat in ((gwq_loc, gwq_g), (uwq_loc, uwq_g), (dwq_loc, dwq_g)):
                nc.gpsimd.collective_compute(
                    "AllGather", ALU.bypass, replica_groups=RG,
                    ins=[loc[:]], outs=[gat[:]],
                )

            # ---- Phase B: quantize x slice, build xqsT via DMA transpose
            with (
                tc.tile_pool(name="xraw", bufs=2) as xraw,
                tc.tile_pool(name="xq", bufs=2) as xqp,
                tc.tile_pool(name="workB", bufs=2) as workB,
            ):
                for tch in range(T_LOC // 128):
                    xt = xraw.tile([128, H], F32, tag="xraw")
                    nc.sync.dma_start(xt[:], x_in[tch * 128:(tch + 1) * 128, :])
                    xq = xqp.tile([128, H], BF16, tag="xq")
                    quantize_tile(nc, workB, xt[:], xq[:], H, gs_x, c448[:])
                    nc.sync.dma_start(xq_d[tch * 128:(tch + 1) * 128, :], xq[:])
                for ht in range(H // 128):
                    nc.sync.dma_start_transpose(
                        xqsT[:, ht, :], xq_d[:, ht * 128:(ht + 1) * 128]
                    )

            # ---- Phase C: gate/up matmuls + SwiGLU + hidden quant
            with (
                tc.tile_pool(name="wstr", bufs=2) as wstr,
                tc.tile_pool(name="psgu", bufs=2, space="PSUM") as psgu,
                tc.tile_pool(name="hwork", bufs=2) as hwork,
                tc.tile_pool(name="workC", bufs=2) as workC,
                tc.tile_pool(name="hqout", bufs=2) as hqout,
            ):
                for ib in range(I // 512):
                    gwT = wstr.tile([128, H // 128, 512], BF16, tag="gwT")
                    uwT = wstr.tile([128, H // 128, 512], BF16, tag="uwT")
                    for ht in range(H // 128):
                        nc.sync.dma_start_transpose(
                            gwT[:, ht, :],
                            gwq_g[ib * 512:(ib + 1) * 512, ht * 128:(ht + 1) * 128],
                        )
                        nc.sync.dma_start_transpose(
                            uwT[:, ht, :],
                            uwq_g[ib * 512:(ib + 1) * 512, ht * 128:(ht + 1) * 128],
                        )
                    for tch in range(T_LOC // 128):
                        pg = psgu.tile([128, 512], F32, tag="pg")
                        pu = psgu.tile([128, 512], F32, tag="pu")
                        for ht in range(H // 128):
                            lhsT = xqsT[:, ht, tch * 128:(tch + 1) * 128]
                            nc.tensor.matmul(
                                pg[:], lhsT, gwT[:, ht, :],
                                start=(ht == 0), stop=(ht == H // 128 - 1),
                            )
                            nc.tensor.matmul(
                                pu[:], lhsT, uwT[:, ht, :],
                                start=(ht == 0), stop=(ht == H // 128 - 1),
                            )
                        sil = hwork.tile([128, 512], F32, tag="sil")
                        nc.scalar.activation(sil[:], pg[:], AF.Silu, scale=s_gate)
                        htr = hwork.tile([128, 512], F32, tag="htr")
                        nc.vector.scalar_tensor_tensor(
                            out=htr[:], in0=sil[:], scalar=s_up, in1=pu[:],
                            op0=ALU.mult, op1=ALU.mult,
                        )
                        hq = hqout.tile([128, 512], BF16, tag="hq")
                        quantize_tile(nc, workC, htr[:], hq[:], 512, gs_h, c448[:])
                        nc.sync.dma_start(
                            hq_d[tch * 128:(tch + 1) * 128, ib * 512:(ib + 1) * 512],
                            hq[:],
                        )

            # ---- Phase D: down matmul + output scale
            with (
                tc.tile_pool(name="dwt", bufs=1) as dwtp,
                tc.tile_pool(name="hqt", bufs=2) as hqtp,
                tc.tile_pool(name="pso", bufs=2, space="PSUM") as pso,
                tc.tile_pool(name="obuf", bufs=2) as obuf,
            ):
                dwT = dwtp.tile([128, I // 128, H], BF16)
                for it in range(I // 128):
                    nc.sync.dma_start_transpose(
                        dwT[:, it, :], dwq_g[:, it * 128:(it + 1) * 128]
                    )
                for tch in range(T_LOC // 128):
                    hqT = hqtp.tile([128, I // 128, 128], BF16, tag="hqT")
                    for it in range(I // 128):
                        nc.sync.dma_start_transpose(
                            hqT[:, it, :],
                            hq_d[tch * 128:(tch + 1) * 128, it * 128:(it + 1) * 128],
                        )
                    po = pso.tile([128, H], F32, tag="po")
                    for it in range(I // 128):
                        lhsT = hqT[:, it, :]
                        nc.tensor.matmul(
                            po[:, 0:512], lhsT, dwT[:, it, 0:512],
                            start=(it == 0), stop=(it == I // 128 - 1),
                        )
                        nc.tensor.matmul(
                            po[:, 512:1024], lhsT, dwT[:, it, 512:1024],
                            start=(it == 0), stop=(it == I // 128 - 1),
                        )
                    ob = obuf.tile([128, H], F32, tag="ob")
                    nc.scalar.activation(ob[:], po[:], AF.Copy, scale=s_down)
                    nc.sync.dma_start(
                        out_d[tch * 128:(tch + 1) * 128, :], ob[:]
                    )

    nc.finalize()
    return nc


_PROG_CACHE = {}
TRACE = False          # set by test.py to capture an NTFF profile
LAST_EXEC_NS = None
LAST_RESULTS = None


def kernel(x, gate_w, up_w, down_w, s_in, s_in_down):
    x = np.ascontiguousarray(x, dtype=np.float32)
    gate_w = np.ascontiguousarray(gate_w, dtype=np.float32)
    up_w = np.ascontiguousarray(up_w, dtype=np.float32)
    down_w = np.ascontiguousarray(down_w, dtype=np.float32)
    gs_x = np.float32(np.asarray(s_in).reshape(-1)[0])
    gs_h = np.float32(np.asarray(s_in_down).reshape(-1)[0])
    FM = np.float32(448.0 * 6.0)
    gs_gw = np.float32(FM / np.abs(gate_w).max())
    gs_uw = np.float32(FM / np.abs(up_w).max())
    gs_dw = np.float32(FM / np.abs(down_w).max())

    key = tuple(float(v) for v in (gs_x, gs_gw, gs_uw, gs_dw, gs_h))
    if key not in _PROG_CACHE:
        _PROG_CACHE.clear()
        _PROG_CACHE[key] = build_program(*key)
    nc = _PROG_CACHE[key]

    xf = x.reshape(T, H)
    in_maps = []
    for c in range(NCORES):
        in_maps.append({
            "x_slice": np.ascontiguousarray(xf[c * T_LOC:(c + 1) * T_LOC]),
            "gw_slice": np.ascontiguousarray(gate_w[c * I_SH:(c + 1) * I_SH]),
            "uw_slice": np.ascontiguousarray(up_w[c * I_SH:(c + 1) * I_SH]),
            "dw_slice": np.ascontiguousarray(down_w[c * HO_SH:(c + 1) * HO_SH]),
        })
    global LAST_EXEC_NS, LAST_RESULTS
    res = run_bass_kernel_spmd(
        nc, in_maps, core_ids=list(range(NCORES)), trace=TRACE
    )
    LAST_EXEC_NS = res.exec_time_ns
    LAST_RESULTS = res
    out = np.concatenate([r["out_slice"] for r in res.results], axis=0)
    return out.reshape(B, S, H).astype(np.float32)


if __name__ == "__main__":
    rng = np.random.default_rng(0)
    inputs = dict(
        x=rng.standard_normal((B, S, H), dtype=np.float32),
        gate_w=0.05 * rng.standard_normal((I, H), dtype=np.float32),
        up_w=0.05 * rng.standard_normal((I, H), dtype=np.float32),
        down_w=0.05 * rng.standard_normal((H, I), dtype=np.float32),
        s_in=np.array([700.0], dtype=np.float32),
        s_in_down=np.array([800.0], dtype=np.float32),
    )
    out = kernel(**inputs)
    print("kernel output", out.shape, out.dtype, np.abs(out).max())

